# revision 13
# baseline (speedup 1.0000x reference)
"""GraphSage 2-layer GNN kernel for Trainium2 (8 NeuronCores, Bass/Tile).

Strategy:
  - Only h1 rows referenced by self_idx2 matter for the output. Each core c
    computes the 1024 output rows h2[c*1024:(c+1)*1024] end-to-end:
    it builds h1 for exactly the segments self_idx2[c*1024:(c+1)*1024]
    references (duplicates computed per-row), so no cross-core collective
    is needed at all.
  - Edges are grouped per output-row block of 128 on the host (CSR sort).
    Segment-sum is done on the TensorEngine with one-hot matmuls in
    *transposed* layout: aggT[d, s] = sum_e msg[e, d] * onehot[e, s],
    which directly produces the lhsT layout the layer matmul needs.
  - Node-feature rows are gathered with indirect DMA (128 rows / inst).
  - Layer matmul: h1T[o, m] = sum_k W1T[k, o].T @ XT[k, m], relu on the
    PSUM->SBUF copy. Layer-2 self features are h1T columns (already in
    SBUF, zero-copy). Final h2T blocks are PE-transposed back to row-major.

Host work is only index/layout preprocessing (argsort + gathers of small
index arrays + edge-feature permutation); all FLOPs + node gathers happen
on device.
"""

import sys

sys.path.insert(0, "/opt/trn_rl_repo")

import numpy as np

from concourse import bacc, mybir, tile
from concourse.bass import IndirectOffsetOnAxis
from concourse.bass_utils import run_bass_kernel_spmd
from concourse.masks import make_identity

# Problem constants (hardcoded per contract)
N, DN, DE, DOUT = 100000, 256, 128, 256
M1, B = 32768, 8192
E1, E2 = 10 * M1, 10 * B
NC = 8
P = 128
R = B // NC            # output rows per core = 1024
NBLK = R // P          # row blocks per core = 8

F32 = mybir.dt.float32
I32 = mybir.dt.int32

_PROGRAM_CACHE = {}


# ---------------------------------------------------------------- host prep
def _csr(seg, nseg):
    perm = np.argsort(seg, kind="stable")
    segs = seg[perm]
    starts = np.searchsorted(segs, np.arange(nseg + 1)).astype(np.int64)
    return perm, starts


def _edges_for(needed, perm, starts):
    """Edge ids (original numbering) for each needed segment, grouped by
    output row; returns (eidx, loc, bcnt): local col 0..127, per-block count."""
    cnt = starts[needed + 1] - starts[needed]
    csum = np.concatenate([[0], np.cumsum(cnt)])
    tot = int(csum[-1])
    pos = np.repeat(starts[needed] - csum[:-1], cnt) + np.arange(tot)
    eidx = perm[pos]
    row = np.repeat(np.arange(needed.shape[0]), cnt)
    loc = (row & (P - 1)).astype(np.float32)
    bcnt = np.bincount(row >> 7, minlength=NBLK).astype(np.int64)
    return eidx, loc, bcnt


def _pad_layer(eidx, loc, bcnt, S, from_idx, ef):
    """Build device arrays for one layer on one core.
    Returns idx [128, NT] i32, segl [128, NT] f32, efp [128, NT*128] f32."""
    NT = NBLK * S
    T = NT * P
    fr = np.zeros(T, np.int32)
    sl = np.full(T, 240.0, np.float32)
    ei = np.zeros(T, np.int64)
    bstart = np.concatenate([[0], np.cumsum(bcnt)])
    for b in range(NBLK):
        n = int(bcnt[b])
        dst = b * S * P
        src = slice(int(bstart[b]), int(bstart[b + 1]))
        ei[dst : dst + n] = eidx[src]
        fr[dst : dst + n] = from_idx[eidx[src]].astype(np.int32)
        sl[dst : dst + n] = loc[src]
    idx_dev = np.ascontiguousarray(fr.reshape(NT, P).T)
    segl_dev = np.ascontiguousarray(sl.reshape(NT, P).T)
    efp = ef[ei]  # [T, DE]
    ef_dev = np.ascontiguousarray(
        efp.reshape(NT, P, DE).transpose(1, 0, 2).reshape(P, NT * DE)
    )
    return idx_dev, segl_dev, ef_dev


def _preprocess(inputs):
    nf = np.asarray(inputs["node_features"], np.float32)
    ef1 = np.asarray(inputs["edge_feats1"], np.float32)
    ef2 = np.asarray(inputs["edge_feats2"], np.float32)
    W1 = np.asarray(inputs["W1"], np.float32)
    W2 = np.asarray(inputs["W2"], np.float32)
    from1 = np.asarray(inputs["from_idx1"], np.int64)
    seg1 = np.asarray(inputs["seg_idx1"], np.int64)
    self1 = np.asarray(inputs["self_idx1"], np.int64)
    from2 = np.asarray(inputs["from_idx2"], np.int64)
    seg2 = np.asarray(inputs["seg_idx2"], np.int64)
    self2 = np.asarray(inputs["self_idx2"], np.int64)

    perm1, starts1 = _csr(seg1, M1)
    perm2, starts2 = _csr(seg2, B)

    per_core = []
    for c in range(NC):
        rows = np.arange(c * R, (c + 1) * R)
        need1 = self2[rows]  # layer-1 segments needed (with dups)
        e1, l1, b1 = _edges_for(need1, perm1, starts1)
        e2, l2, b2 = _edges_for(rows, perm2, starts2)
        per_core.append((need1, e1, l1, b1, e2, l2, b2))

    S1 = max(1, int(max(int(np.ceil(pc[3].max() / P)) for pc in per_core)))
    S2 = max(1, int(max(int(np.ceil(pc[6].max() / P)) for pc in per_core)))

    w1t = np.ascontiguousarray(W1.T)  # [640, 256]
    w2t = np.ascontiguousarray(W2.T)
    iota = np.tile(np.arange(P, dtype=np.float32), (P, 1))  # [128,128], [p,f]=f

    in_maps = []
    for c in range(NC):
        need1, e1, l1, b1, e2, l2, b2 = per_core[c]
        idx1, segl1, efp1 = _pad_layer(e1, l1, b1, S1, from1, ef1)
        idx2, segl2, efp2 = _pad_layer(e2, l2, b2, S2, from2, ef2)
        sidx1 = np.ascontiguousarray(
            self1[need1].astype(np.int32).reshape(NBLK, P).T
        )  # [128, NBLK]
        in_maps.append(
            {
                "nf": nf,
                "efp1": efp1,
                "efp2": efp2,
                "idx1": idx1,
                "idx2": idx2,
                "segl1": segl1,
                "segl2": segl2,
                "sidx1": sidx1,
                "w1t": w1t,
                "w2t": w2t,
                "iota": iota,
            }
        )
    return in_maps, S1, S2


# ---------------------------------------------------------------- device prog
def _build_program(S1, S2, dbg=False):
    key = (S1, S2, dbg)
    if key in _PROGRAM_CACHE:
        return _PROGRAM_CACHE[key]

    nc = bacc.Bacc(
        "TRN2", target_bir_lowering=False, debug=False, num_devices=NC
    )

    nf_h = nc.dram_tensor("nf", [N, DN], F32, kind="ExternalInput")
    efp1_h = nc.dram_tensor("efp1", [P, NBLK * S1 * DE], F32, kind="ExternalInput")
    efp2_h = nc.dram_tensor("efp2", [P, NBLK * S2 * DE], F32, kind="ExternalInput")
    idx1_h = nc.dram_tensor("idx1", [P, NBLK * S1], I32, kind="ExternalInput")
    idx2_h = nc.dram_tensor("idx2", [P, NBLK * S2], I32, kind="ExternalInput")
    segl1_h = nc.dram_tensor("segl1", [P, NBLK * S1], F32, kind="ExternalInput")
    segl2_h = nc.dram_tensor("segl2", [P, NBLK * S2], F32, kind="ExternalInput")
    sidx1_h = nc.dram_tensor("sidx1", [P, NBLK], I32, kind="ExternalInput")
    w1t_h = nc.dram_tensor("w1t", [2 * DN + DE, DOUT], F32, kind="ExternalInput")
    w2t_h = nc.dram_tensor("w2t", [2 * DN + DE, DOUT], F32, kind="ExternalInput")
    iota_h = nc.dram_tensor("iota", [P, P], F32, kind="ExternalInput")
    out_h = nc.dram_tensor("out", [R, DOUT], F32, kind="ExternalOutput")
    if dbg:
        dbg_g = nc.dram_tensor("dbg_g", [P, DN], F32, kind="ExternalOutput")
        dbg_oh = nc.dram_tensor("dbg_oh", [P, P], F32, kind="ExternalOutput")
        dbg_agg = nc.dram_tensor("dbg_agg", [P, 3 * P], F32, kind="ExternalOutput")
        dbg_xt = nc.dram_tensor("dbg_xt", [P, 5 * P], F32, kind="ExternalOutput")
        dbg_h1 = nc.dram_tensor("dbg_h1", [2 * P, R], F32, kind="ExternalOutput")

    nf_ap = nf_h.ap()
    KT = (2 * DN + DE) // P  # 5 contraction tiles of 128

    with tile.TileContext(nc) as tc:
        with (
            tc.tile_pool(name="const", bufs=1) as constp,
            tc.tile_pool(name="ef", bufs=1) as efpool,
            tc.tile_pool(name="h1", bufs=1) as h1pool,
            tc.tile_pool(name="gat", bufs=6) as gatp,
            tc.tile_pool(name="oh", bufs=6) as ohp,
            tc.tile_pool(name="xt", bufs=3) as xtp,
            tc.tile_pool(name="orow", bufs=2) as orowp,
            tc.tile_pool(name="pagg", bufs=1, space="PSUM") as paggp,
            tc.tile_pool(name="pmm", bufs=1, space="PSUM") as pmmp,
            tc.tile_pool(name="ptr", bufs=1, space="PSUM") as ptrp,
        ):
            # constants
            iota_sb = constp.tile([P, P], F32, tag="iota")
            nc.sync.dma_start(out=iota_sb[:], in_=iota_h.ap())
            ident = constp.tile([P, P], F32, tag="ident")
            make_identity(nc, ident[:])

            w_sb = []
            for li, wh in ((0, w1t_h), (1, w2t_h)):
                wt = constp.tile([P, KT * DOUT], F32, tag=f"w{li}")
                for kt in range(KT):
                    nc.sync.dma_start(
                        out=wt[:, kt * DOUT : (kt + 1) * DOUT],
                        in_=wh.ap()[kt * P : (kt + 1) * P, :],
                    )
                w_sb.append(wt)

            idx_sb, segl_sb, ef_sb = [], [], []
            for li, (S, ih, sh, eh) in enumerate(
                ((S1, idx1_h, segl1_h, efp1_h), (S2, idx2_h, segl2_h, efp2_h))
            ):
                NT = NBLK * S
                it = constp.tile([P, NT], I32, tag=f"idx{li}")
                nc.sync.dma_start(out=it[:], in_=ih.ap())
                st = constp.tile([P, NT], F32, tag=f"segl{li}")
                nc.sync.dma_start(out=st[:], in_=sh.ap())
                et = efpool.tile([P, NT * DE], F32, tag=f"ef{li}")
                nc.sync.dma_start(out=et[:], in_=eh.ap())
                idx_sb.append(it)
                segl_sb.append(st)
                ef_sb.append(et)

            sidx_sb = constp.tile([P, NBLK], I32, tag="sidx")
            nc.sync.dma_start(out=sidx_sb[:], in_=sidx1_h.ap())

            h1T = [
                h1pool.tile([P, R], F32, tag=f"h1T{o}", name=f"h1T{o}")
                for o in range(2)
            ]

            for layer in range(2):
                S = (S1, S2)[layer]
                for b in range(NBLK):
                    # ---- aggregate messages into aggT psum, one bank per
                    # accumulation group (start=True resets at bank scope)
                    agg_ps = [
                        paggp.tile([P, P], F32, tag=f"agg{d}", name=f"agg{d}")
                        for d in range(3)
                    ]
                    for t in range(S):
                        tg = b * S + t
                        g = gatp.tile([P, DN], F32, tag="gat")
                        nc.gpsimd.indirect_dma_start(
                            out=g[:],
                            out_offset=None,
                            in_=nf_ap,
                            in_offset=IndirectOffsetOnAxis(
                                ap=idx_sb[layer][:, tg : tg + 1], axis=0
                            ),
                        )
                        oh = ohp.tile([P, P], F32, tag="oh")
                        nc.vector.tensor_scalar(
                            out=oh[:],
                            in0=iota_sb[:],
                            scalar1=segl_sb[layer][:, tg : tg + 1],
                            scalar2=None,
                            op0=mybir.AluOpType.is_equal,
                        )
                        if dbg and layer == 0 and b == 0 and t == 0:
                            nc.sync.dma_start(out=dbg_g.ap(), in_=g[:])
                            nc.sync.dma_start(out=dbg_oh.ap(), in_=oh[:])
                        st_flag = t == 0
                        sp_flag = t == S - 1
                        nc.tensor.matmul(
                            agg_ps[0][:],
                            lhsT=g[:, 0:P],
                            rhs=oh[:],
                            start=st_flag,
                            stop=sp_flag,
                        )
                        nc.tensor.matmul(
                            agg_ps[1][:],
                            lhsT=g[:, P : 2 * P],
                            rhs=oh[:],
                            start=st_flag,
                            stop=sp_flag,
                        )
                        nc.tensor.matmul(
                            agg_ps[2][:],
                            lhsT=ef_sb[layer][:, tg * DE : (tg + 1) * DE],
                            rhs=oh[:],
                            start=st_flag,
                            stop=sp_flag,
                        )

                    # ---- build XT [k=640, m=128]; k0,k1 = self features^T
                    xt = xtp.tile([P, KT * P], F32, tag="xt")
                    if layer == 0:
                        sg = gatp.tile([P, DN], F32, tag="sgat")
                        nc.gpsimd.indirect_dma_start(
                            out=sg[:],
                            out_offset=None,
                            in_=nf_ap,
                            in_offset=IndirectOffsetOnAxis(
                                ap=sidx_sb[:, b : b + 1], axis=0
                            ),
                        )
                        tp = ptrp.tile([P, DN], F32, tag="tr")
                        nc.tensor.transpose(tp[:, 0:P], sg[:, 0:P], ident[:])
                        nc.tensor.transpose(tp[:, P : 2 * P], sg[:, P : 2 * P], ident[:])
                        nc.vector.tensor_copy(out=xt[:, 0 : 2 * P], in_=tp[:])
                    else:
                        nc.vector.tensor_copy(
                            out=xt[:, 0:P], in_=h1T[0][:, b * P : (b + 1) * P]
                        )
                        nc.vector.tensor_copy(
                            out=xt[:, P : 2 * P], in_=h1T[1][:, b * P : (b + 1) * P]
                        )
                    for d in range(3):
                        nc.vector.tensor_copy(
                            out=xt[:, (2 + d) * P : (3 + d) * P], in_=agg_ps[d][:]
                        )
                    if dbg and layer == 0 and b == 0:
                        nc.sync.dma_start(out=dbg_agg.ap(), in_=xt[:, 2 * P : KT * P])
                        nc.sync.dma_start(out=dbg_xt.ap(), in_=xt[:])

                    # ---- layer matmul: hT[o, m] += W_T[k, o].T @ XT[k, m]
                    mm = [
                        pmmp.tile([P, P], F32, tag=f"mm{o}", name=f"mm{o}")
                        for o in range(2)
                    ]
                    for ot in range(2):
                        for kt in range(KT):
                            nc.tensor.matmul(
                                mm[ot][:],
                                lhsT=w_sb[layer][
                                    :, kt * DOUT + ot * P : kt * DOUT + (ot + 1) * P
                                ],
                                rhs=xt[:, kt * P : (kt + 1) * P],
                                start=kt == 0,
                                stop=kt == KT - 1,
                            )

                    if layer == 0:
                        for ot in range(2):
                            nc.scalar.activation(
                                h1T[ot][:, b * P : (b + 1) * P],
                                mm[ot][:],
                                mybir.ActivationFunctionType.Relu,
                            )
                        if dbg and b == NBLK - 1:
                            nc.sync.dma_start(out=dbg_h1.ap()[0:P, :], in_=h1T[0][:])
                            nc.sync.dma_start(
                                out=dbg_h1.ap()[P : 2 * P, :], in_=h1T[1][:]
                            )
                    else:
                        rl = orowp.tile([P, 2 * P], F32, tag="rl")
                        for ot in range(2):
                            nc.scalar.activation(
                                rl[:, ot * P : (ot + 1) * P],
                                mm[ot][:],
                                mybir.ActivationFunctionType.Relu,
                            )
                        tp2 = ptrp.tile([P, 2 * P], F32, tag="tr2")
                        nc.tensor.transpose(tp2[:, 0:P], rl[:, 0:P], ident[:])
                        nc.tensor.transpose(tp2[:, P : 2 * P], rl[:, P : 2 * P], ident[:])
                        orow = orowp.tile([P, 2 * P], F32, tag="orow")
                        nc.vector.tensor_copy(out=orow[:], in_=tp2[:])
                        nc.sync.dma_start(
                            out=out_h.ap()[b * P : (b + 1) * P, :], in_=orow[:]
                        )

    nc.compile()
    _PROGRAM_CACHE[key] = nc
    return nc


# ---------------------------------------------------------------- entry point
def kernel(**inputs) -> np.ndarray:
    in_maps, S1, S2 = _preprocess(inputs)
    nc = _build_program(S1, S2)
    res = run_bass_kernel_spmd(nc, in_maps, list(range(NC)))
    return np.concatenate([r["out"] for r in res.results], axis=0)
